# revision 40
# baseline (speedup 1.0000x reference)
"""Trainium2 Bass kernel for nn_CausalSE (chunked-EMA squeeze-excite gating).

Reference computation (per batch b):
    xc   = mean over chunks of 16 along L            -> [C, N]   (N = L/16)
    e_t  = g*e_{t-1} + (1-g)*xc_t   (causal EMA)     -> [C, N]
    h    = relu(w1 @ e + b1)                         -> [C/8, N]
    gate = sigmoid(w2 @ h + b2)                      -> [C, N]
    out  = repeat(gate, 16) * x                      -> [C, L]

Distribution: pure data-parallel over batch. B == 8 == n_cores, each core
processes one full batch element independently; no collectives.

Numerics (budget 2e-2, we spend ~0.3%): x and out move as bf16 (halves
the HBM traffic, which at ~384 GB/s/core mixed is the roofline: 16.8 MB
-> ~44 us).  The EMA runs on pooled sums with ((1-gamma)/16) folded into
w1 on the host; gamma stays f32.

Engine placement (per column chunk; measured on HW, see below):
  SP (sync)    : in-DMA triggers ONLY — a dedicated input queue.  Putting
                 out-triggers here head-of-line-blocks the input stream
                 behind the gating multiply (+4..10us measured).
  DVE (vector) : pooling tree (bf16 2x packed) + EMA scans (the scan
                 opcode only exists on DVE) + gating multiply (2x).  This
                 queue paces the kernel end-to-end (~38us busy).
  Act (scalar) : relu, sigmoid, the whole gate expansion (gpsimd copies
                 measured 4x slower than the cost model claims).
  Pool (gpsimd): out-DMA triggers ONLY (SWDGE) — dedicated output queue.
  PE (tensor)  : SE bottleneck matmuls in bf16.
The gating multiply + out-DMA of chunk k are emitted `skew` chunks later
so the DVE queue never stalls waiting on the SE chain, and each chunk is
split in ct-halves (gsplit=2) so outs start as soon as half the gating
is done.

HW-measured context (8 cores concurrent, per core): mixed in+out DMA
streams at ~384 GB/s -> 43.7us floor for 16.8 MB; this kernel runs
~58us single-shot (honest ping-pong-reps timing; the naive same-address
reps measurement dead-store-eliminates the out-DMAs and reads ~15% fast).

Timing-graph liveness: reps>1 graphs "ping-pong" the I/O (rep 0 reads
x writes out, rep 1 reads out writes out2, ...) so every rep's transfers
and compute are data-live — otherwise the compiler dead-store-eliminates
repeated same-address out-DMAs and the reps-slope undercounts.
"""

import numpy as np
from contextlib import ExitStack

import concourse.bass as bass
import concourse.tile as tile
from concourse import bacc, mybir

F32 = mybir.dt.float32
BF16 = mybir.dt.bfloat16
P = 128


def _bcast_ap(ap, n, drop_last=False):
    """Append a stride-0 dim of size n to an AP (optionally replacing a
    trailing size-1 dim)."""
    dims = [list(d) for d in ap.ap]
    if drop_last:
        assert dims[-1][1] == 1, dims
        dims = dims[:-1]
    dims = dims + [[0, n]]
    return bass.AP(tensor=ap.tensor, offset=ap.offset, ap=dims)


def build_graph(C=512, L=8192, CS=16, HID=64, reps=1, chunks=None,
                scan_eng="gpsimd", exp_act=3, out_eng="sync", skew=2,
                gsplit=1, jslab=16, pingpong=False, serialize=False,
                ablate=None, xbufs=1, sbufs=3, ebufs=4, pbufs=2):
    """Build the per-core Bass graph (SPMD: every core runs this same graph).

    chunks : column widths (each a multiple of CS, sum == L).
    exp_act: how many of the NCT channel tiles get their gate expansion on
             the Act engine (the rest go on Pool/gpsimd).
    out_eng: engine issuing the out-DMA triggers ("sync"|"scalar").
    pingpong: reps>1 timing graphs alternate DRAM src/dst so all work is
             live (see module docstring).  reps==1 is the real kernel.
    serialize: chain rep r's first in-DMA after rep r-1's last out-DMA
             (single-shot latency instead of pipelined throughput).
    """
    NCT = C // P
    if chunks is None:
        chunks = [512, 1536, 2048, 2048, 1536, 512]
    assert sum(chunks) == L and all(c % CS == 0 for c in chunks)
    NCmax = max(chunks) // CS
    LCmax = max(chunks)

    nc = bacc.Bacc(None, target_bir_lowering=False)

    x_ext = nc.declare_dram_parameter("x", [C, L], BF16, isOutput=False)
    w1_ext = nc.declare_dram_parameter("w1s", [P, NCT * HID], BF16, isOutput=False)
    w2_ext = nc.declare_dram_parameter("w2t", [HID, C], BF16, isOutput=False)
    b1_ext = nc.declare_dram_parameter("b1", [HID, 1], F32, isOutput=False)
    g_ext = nc.declare_dram_parameter("g", [P, NCT], F32, isOutput=False)
    out_ext = nc.declare_dram_parameter("out", [C, L], BF16, isOutput=True)
    dbg_ext = None
    if ablate == "no_gate":
        # keep the SE chain live without the expansion/multiply consumers
        dbg_ext = nc.declare_dram_parameter(
            "dbg", [P, reps * len(chunks) * NCT * NCmax], BF16, isOutput=True)
    views = [x_ext[:].rearrange("(ct p) l -> p ct l", ct=NCT),
             out_ext[:].rearrange("(ct p) l -> p ct l", ct=NCT)]
    if pingpong and reps > 1:
        out2_ext = nc.declare_dram_parameter("out2", [C, L], BF16, isOutput=True)
        views.append(out2_ext[:].rearrange("(ct p) l -> p ct l", ct=NCT))

    from concourse.tile_rust import add_dep_helper

    with ExitStack() as ctx:
        tc = ctx.enter_context(tile.TileContext(nc))
        consts = ctx.enter_context(tc.tile_pool(name="consts", bufs=1))
        xpool = ctx.enter_context(tc.tile_pool(name="xpool", bufs=xbufs))
        small = ctx.enter_context(tc.tile_pool(name="small", bufs=sbufs))
        epool = ctx.enter_context(tc.tile_pool(name="epool", bufs=ebufs))
        psum = ctx.enter_context(
            tc.tile_pool(name="psum", bufs=pbufs, space=bass.MemorySpace.PSUM)
        )

        # -- head: warm the ACT function tables, load the small constants --
        warm = consts.tile([P, 1], F32, name="warm")
        nc.gpsimd.memset(warm[:], 0.0)
        nc.scalar.activation(out=warm[:], in_=warm[:],
                             func=mybir.ActivationFunctionType.Sigmoid)
        nc.scalar.activation(out=warm[:], in_=warm[:],
                             func=mybir.ActivationFunctionType.Relu)

        g_sb = consts.tile([P, NCT], F32, name="g_sb")
        nc.scalar.dma_start(out=g_sb[:], in_=g_ext[:])
        w1_sb = consts.tile([P, NCT, HID], BF16)
        nc.scalar.dma_start(
            out=w1_sb[:], in_=w1_ext[:].rearrange("p (ct h) -> p ct h", ct=NCT)
        )
        w2_sb = consts.tile([HID, C], BF16)
        nc.scalar.dma_start(out=w2_sb[:], in_=w2_ext[:])
        b1_sb = consts.tile([HID, 1], F32)
        nc.scalar.dma_start(out=b1_sb[:], in_=b1_ext[:])

        # materialized per-ct gamma rows for the scan's data0 operand
        ones = consts.tile([P, NCmax], F32)
        nc.vector.memset(ones[:], 1.0)
        g_bcast = []
        for ct in range(NCT):
            gb = consts.tile([P, NCmax], F32, tag=f"gb{ct}")
            nc.vector.tensor_scalar_mul(gb[:], ones[:], g_sb[:, ct : ct + 1])
            g_bcast.append(gb)

        scan_e = {"gpsimd": nc.gpsimd, "vector": nc.vector}[scan_eng]
        out_engines = {"sync": [nc.sync], "scalar": [nc.scalar],
                       "gpsimd": [nc.gpsimd],
                       "alt": [nc.gpsimd, nc.scalar]}[out_eng]
        out_idx = [0]

        const_gate = None
        if ablate == "no_se":
            const_gate = consts.tile([P, NCT, LCmax], BF16, name="cgate")
            nc.vector.memset(const_gate[:], 0.5)
        dbg_i = 0

        prev_rep_last_out = None
        for _r in range(reps):
            src = views[0] if _r == 0 else views[1 + ((_r - 1) % 2)]
            dst = views[1 + (_r % 2)] if (pingpong and reps > 1) else views[1]
            last_out_inst = None
            # chunk-block-major resident x: each chunk's [ct, LC] block is a
            # CONTIGUOUS column range, so Tile's byte-range hazard tracking
            # sees the chunks as disjoint and the pipeline stays overlapped.
            x_big = xpool.tile([P, NCT * L], BF16, tag="xbig", name="xbig")
            u_prev = None
            prev_nc = 0
            col = 0
            off = 0
            pending = []  # deferred gating multiplies / out-DMAs (skewed)

            def emit_gate_apply(poff, pcol, pLC, gate_exp):
                nonlocal last_out_inst
                xblk = x_big[:, poff : poff + NCT * pLC].rearrange(
                    "p (ct l) -> p ct l", ct=NCT
                )
                gw = NCT // gsplit
                pNC = pLC // CS
                for g in range(gsplit):
                    c0, c1 = g * gw, (g + 1) * gw
                    if jslab == CS:
                        nc.vector.tensor_tensor(
                            out=xblk[:, c0:c1, :],
                            in0=xblk[:, c0:c1, :],
                            in1=gate_exp[:, c0:c1, :pLC],
                            op=mybir.AluOpType.mult,
                        )
                    else:
                        # gate expanded only to j-slabs of width jslab; the
                        # multiply walks the CS/jslab phases (all 2x packed)
                        x4 = xblk[:, c0:c1, :].rearrange(
                            "p ct (n j) -> p ct n j", j=CS
                        )
                        g4 = gate_exp[:, c0:c1, : pNC * jslab].rearrange(
                            "p ct (n j) -> p ct n j", j=jslab
                        )
                        for j0 in range(0, CS, jslab):
                            nc.vector.tensor_tensor(
                                out=x4[:, :, :, j0 : j0 + jslab],
                                in0=x4[:, :, :, j0 : j0 + jslab],
                                in1=g4,
                                op=mybir.AluOpType.mult,
                            )
                    last_out_inst = out_engines[
                        out_idx[0] % len(out_engines)
                    ].dma_start(
                        out=dst[:, c0:c1, pcol : pcol + pLC],
                        in_=xblk[:, c0:c1, :],
                    )
                    out_idx[0] += 1

            for k, LC in enumerate(chunks):
                NCc = LC // CS
                xblk = x_big[:, off : off + NCT * LC].rearrange(
                    "p (ct l) -> p ct l", ct=NCT
                )
                in_inst = nc.sync.dma_start(
                    out=xblk, in_=src[:, :, col : col + LC]
                )
                if serialize and k == 0 and prev_rep_last_out is not None:
                    add_dep_helper(
                        in_inst.ins,
                        prev_rep_last_out.ins,
                        reason="serialize reps for single-shot timing",
                    )

                if ablate == "no_se":
                    pending.append((off, col, LC, const_gate))
                    if len(pending) > skew:
                        emit_gate_apply(*pending.pop(0))
                    col += LC
                    off += NCT * LC
                    continue

                # fused pooling as a pairwise-halving tree: each stage sums
                # the two halves of every group, with contiguous >=2-element
                # runs so the DVE's 2x packed mode applies.
                G = NCT * NCc
                stage_src = x_big[:, off : off + NCT * LC]
                r = CS
                while r > 1:
                    h = r // 2
                    v = stage_src.rearrange("p (g j) -> p g j", j=r)
                    dt_ = small.tile([P, NCT * NCmax * h], BF16, tag=f"pool{h}")
                    nc.vector.tensor_tensor(
                        out=dt_[:, : G * h].rearrange("p (g j) -> p g j", j=h),
                        in0=v[:, :, :h],
                        in1=v[:, :, h:],
                        op=mybir.AluOpType.add,
                    )
                    stage_src = dt_[:, : G * h]
                    r = h
                xc_all = stage_src.rearrange("p (ct n) -> p ct n", ct=NCT)

                # causal EMA over chunk steps (per ct; serial across chunks)
                u_all = small.tile([P, NCT, NCmax], BF16, tag="u")
                for ct in range(NCT):
                    init = (
                        0.0 if k == 0
                        else u_prev[:, ct, prev_nc - 1 : prev_nc]
                    )
                    scan_e.tensor_tensor_scan(
                        out=u_all[:, ct, :NCc],
                        data0=g_bcast[ct][:, :NCc],
                        data1=xc_all[:, ct, :NCc],
                        initial=init,
                        op0=mybir.AluOpType.mult,
                        op1=mybir.AluOpType.add,
                    )

                # SE bottleneck
                h_ps = psum.tile([HID, NCmax], F32, tag="hps")
                for ct in range(NCT):
                    nc.tensor.matmul(
                        h_ps[:, :NCc],
                        w1_sb[:, ct, :],
                        u_all[:, ct, :NCc],
                        start=(ct == 0),
                        stop=(ct == NCT - 1),
                    )
                h_sb = small.tile([HID, NCmax], BF16, tag="h")
                nc.scalar.activation(
                    out=h_sb[:, :NCc],
                    in_=h_ps[:, :NCc],
                    func=mybir.ActivationFunctionType.Relu,
                    bias=b1_sb[:],
                )
                o_all = psum.tile([P, NCT * NCmax], F32, tag="ops")
                for ct in range(NCT):
                    nc.tensor.matmul(
                        o_all[:, ct * NCc : (ct + 1) * NCc],
                        w2_sb[:, ct * P : (ct + 1) * P],
                        h_sb[:, :NCc],
                        start=True,
                        stop=True,
                    )
                gate_all = small.tile([P, NCT, NCmax], BF16, tag="gate")
                o_v = o_all[:, : NCT * NCc].rearrange("p (ct n) -> p ct n", ct=NCT)
                nc.scalar.activation(
                    out=gate_all[:, :, :NCc],
                    in_=o_v,
                    func=mybir.ActivationFunctionType.Sigmoid,
                )

                if ablate == "no_gate":
                    # ship the gate to a debug output instead of gating
                    nc.scalar.dma_start(
                        out=dbg_ext[:, dbg_i : dbg_i + NCT * NCmax],
                        in_=gate_all[:, :, :].rearrange("p ct n -> p (ct n)"),
                    )
                    dbg_i += NCT * NCmax
                    u_prev = u_all
                    prev_nc = NCc
                    col += LC
                    off += NCT * LC
                    continue

                # expand gate [p, ct, n] -> [p, ct, n*jslab] (j-slab; the
                # multiply re-reads the slab for each of the CS/jslab phases)
                gate_exp = epool.tile([P, NCT, NCmax * jslab], BF16, tag="gexp")
                if gsplit > 1:
                    gw = NCT // gsplit
                    for g in range(gsplit):
                        c0, c1 = g * gw, (g + 1) * gw
                        nc.scalar.copy(
                            out=gate_exp[:, c0:c1, : NCc * jslab].rearrange(
                                "p ct (n j) -> p ct n j", j=jslab
                            ),
                            in_=_bcast_ap(gate_all[:, c0:c1, :NCc], jslab),
                        )
                else:
                    na = min(exp_act, NCT)
                    if na > 0:
                        nc.scalar.copy(
                            out=gate_exp[:, :na, :LC].rearrange(
                                "p ct (n j) -> p ct n j", j=CS
                            ),
                            in_=_bcast_ap(gate_all[:, :na, :NCc], CS),
                        )
                    if na < NCT:
                        nc.gpsimd.tensor_copy(
                            gate_exp[:, na:, :LC].rearrange(
                                "p ct (n j) -> p ct n j", j=CS
                            ),
                            _bcast_ap(gate_all[:, na:, :NCc], CS),
                        )

                pending.append((off, col, LC, gate_exp))
                skew_k = skew[k] if isinstance(skew, (list, tuple)) else skew
                while len(pending) > skew_k:
                    emit_gate_apply(*pending.pop(0))

                u_prev = u_all
                prev_nc = NCc
                col += LC
                off += NCT * LC

            while pending:
                emit_gate_apply(*pending.pop(0))
            prev_rep_last_out = last_out_inst

    nc.compile()
    return nc


def _f32_to_bf16(a):
    """Round-to-nearest-even f32 -> bf16, vectorized (no NaN handling)."""
    import ml_dtypes

    u = np.ascontiguousarray(a, np.float32).view(np.uint32)
    r = ((u + 0x7FFF + ((u >> 16) & 1)) >> 16).astype(np.uint16)
    return r.view(ml_dtypes.bfloat16)


def _bf16_to_f32(a):
    u = np.asarray(a).view(np.uint16).astype(np.uint32) << 16
    return u.view(np.float32)


def host_prep(gamma, w1, b1, w2, b2, C=512, HID=64):
    """Host-side preprocessing of the shared (small) tensors."""
    NCT = C // P
    gamma = np.asarray(gamma, np.float32)
    w1 = np.asarray(w1, np.float32)
    w2 = np.asarray(w2, np.float32)
    bv = (1.0 - gamma) / 16.0
    w1s = (w1 * bv[None, :]).T  # [C, HID]
    w1s_r = _f32_to_bf16(
        np.ascontiguousarray(
            w1s.reshape(NCT, P, HID).transpose(1, 0, 2).reshape(P, NCT * HID)
        )
    )
    w2t = _f32_to_bf16(np.ascontiguousarray(w2.T))  # [HID, C]
    b1_r = np.ascontiguousarray(np.asarray(b1, np.float32).reshape(HID, 1))
    g_r = np.ascontiguousarray(gamma.reshape(NCT, P).T)
    return w1s_r, w2t, b1_r, g_r


DEFAULT_CFG = dict(
    chunks=[1024, 2048, 2048, 2048, 768, 256],
    scan_eng="vector",
    exp_act=4,
    out_eng="gpsimd",
    # chunk k's gating multiply + out-DMA are deferred until the pending
    # queue exceeds skew[k]: chunk 0 flushes right after chunk 1 (its
    # expansion is ready by then with jslab=4, filling the DVE input-wait
    # gap at the head), later chunks run 2 behind.
    skew=[9, 1, 2, 2, 2, 2],
    gsplit=2,
    # gate expanded only to j-slabs of 4 (the multiply walks 4 phases,
    # all 2x packed): cuts the Act expansion pass from 27us to 6.8us so
    # the multiply never waits on it.
    jslab=4,
    ebufs=5,
)

_GRAPH_CACHE = {}


def _get_graph(reps=1):
    key = reps
    if key not in _GRAPH_CACHE:
        _GRAPH_CACHE[key] = build_graph(reps=reps, **DEFAULT_CFG)
    return _GRAPH_CACHE[key]


def make_in_maps(x, gamma, w1, b1, w2, b2):
    B, C, L = x.shape
    HID = w1.shape[0]
    w1s_r, w2t, b1_r, g_r = host_prep(gamma, w1, b1, w2, b2, C=C, HID=HID)
    xb = _f32_to_bf16(x).reshape(B, C, L)
    return [
        {
            "x": xb[b],  # view of the contiguous parent -> no copy downstream
            "w1s": w1s_r,
            "w2t": w2t,
            "b1": b1_r,
            "g": g_r,
        }
        for b in range(B)
    ]


_RUNNER_CACHE = {}


def _make_runner(nc, n_cores):
    """Persistent jitted SPMD runner for `nc` across `n_cores` devices.

    Returns run(in_maps) -> list[dict] of per-core outputs.
    """
    import jax
    from jax.sharding import Mesh, PartitionSpec
    from jax.experimental.shard_map import shard_map
    from concourse import bass2jax

    bass2jax.install_neuronx_cc_hook()

    partition_name = nc.partition_id_tensor.name if nc.partition_id_tensor else None
    in_names, out_names, out_avals = [], [], []
    for alloc in nc.m.functions[0].allocations:
        if not isinstance(alloc, mybir.MemoryLocationSet):
            continue
        name = alloc.memorylocations[0].name
        if alloc.kind == "ExternalInput":
            if name != partition_name:
                in_names.append(name)
        elif alloc.kind == "ExternalOutput":
            out_names.append(name)
            out_avals.append(
                jax.core.ShapedArray(tuple(alloc.tensor_shape), mybir.dt.np(alloc.dtype))
            )
    n_params = len(in_names)
    in_names_all = in_names + out_names
    if partition_name is not None:
        in_names_all.append(partition_name)

    def _body(*args):
        operands = list(args)
        if partition_name is not None:
            operands.append(bass2jax.partition_id_tensor())
        outs = bass2jax._bass_exec_p.bind(
            *operands,
            out_avals=tuple(out_avals),
            in_names=tuple(in_names_all),
            out_names=tuple(out_names),
            lowering_input_output_aliases=(),
            sim_require_finite=True,
            sim_require_nnan=True,
            nc=nc,
        )
        return tuple(outs)

    devices = jax.devices()[:n_cores]
    mesh = Mesh(np.asarray(devices), ("core",))
    n_outs = len(out_avals)
    sharded = jax.jit(
        shard_map(
            _body,
            mesh=mesh,
            in_specs=(PartitionSpec("core"),) * (n_params + n_outs),
            out_specs=(PartitionSpec("core"),) * len(out_names),
            check_rep=False,
        ),
        keep_unused=True,
    )
    concat_zeros = [
        np.zeros((n_cores * a.shape[0], *a.shape[1:]), a.dtype) for a in out_avals
    ]

    def _concat_inputs(in_maps):
        concat_in = []
        for name in in_names:
            parts = [np.asarray(m[name]) for m in in_maps]
            base = parts[0].base if parts[0].base is not None else parts[0]
            if (
                base.ndim == parts[0].ndim + 1
                and base.shape[0] == n_cores
                and base.flags.c_contiguous
                and all(
                    p.base is base
                    and p.__array_interface__["data"][0]
                    == base.__array_interface__["data"][0] + c * parts[0].nbytes
                    for c, p in enumerate(parts)
                )
            ):
                # per-core slices of one contiguous parent: reshape, no copy
                concat_in.append(
                    np.ascontiguousarray(base).reshape(
                        n_cores * parts[0].shape[0], *parts[0].shape[1:]
                    )
                )
            else:
                concat_in.append(np.concatenate(parts, axis=0))
        return concat_in

    def run(in_maps):
        out_arrs = sharded(*_concat_inputs(in_maps), *concat_zeros)
        return [
            {
                name: np.asarray(out_arrs[i]).reshape(
                    n_cores, *out_avals[i].shape
                )[c]
                for i, name in enumerate(out_names)
            }
            for c in range(n_cores)
        ]

    def run_full(in_maps):
        """Like run() but returns the first output as one stacked array
        [n_cores, ...] with a single host copy."""
        out_arrs = sharded(*_concat_inputs(in_maps), *concat_zeros)
        return np.asarray(out_arrs[0]).reshape(n_cores, *out_avals[0].shape)

    run.run_full = run_full
    return run


def _get_runner(reps=1, n_cores=8):
    key = (reps, n_cores)
    if key not in _RUNNER_CACHE:
        _RUNNER_CACHE[key] = _make_runner(_get_graph(reps=reps), n_cores)
    return _RUNNER_CACHE[key]


def kernel(x, gamma, w1, b1, w2, b2):
    x = np.asarray(x)
    B, C, L = x.shape
    assert (B, C, L) == (8, 512, 8192), (B, C, L)
    in_maps = make_in_maps(x, gamma, w1, b1, w2, b2)
    try:
        out = _get_runner(reps=1, n_cores=B).run_full(in_maps)
        return np.ascontiguousarray(_bf16_to_f32(out))
    except Exception:
        # fallback: the official (slower to dispatch, identical NEFF) path
        from concourse.bass_utils import run_bass_kernel_spmd

        res = run_bass_kernel_spmd(
            _get_graph(reps=1), in_maps, core_ids=list(range(B))
        ).results
        out = np.stack([res[b]["out"] for b in range(B)], axis=0)
        return np.ascontiguousarray(_bf16_to_f32(out))


# revision 45
# speedup vs baseline: 1.0205x; 1.0205x over previous
"""Trainium2 Bass kernel for nn_CausalSE (chunked-EMA squeeze-excite gating).

Reference computation (per batch b):
    xc   = mean over chunks of 16 along L            -> [C, N]   (N = L/16)
    e_t  = g*e_{t-1} + (1-g)*xc_t   (causal EMA)     -> [C, N]
    h    = relu(w1 @ e + b1)                         -> [C/8, N]
    gate = sigmoid(w2 @ h + b2)                      -> [C, N]
    out  = repeat(gate, 16) * x                      -> [C, L]

Distribution: pure data-parallel over batch. B == 8 == n_cores, each core
processes one full batch element independently; no collectives.

Numerics (budget 2e-2, we spend ~0.3%): x and out move as bf16 (halves
the HBM traffic, which at ~384 GB/s/core mixed is the roofline: 16.8 MB
-> ~44 us).  The EMA runs on pooled sums with ((1-gamma)/16) folded into
w1 on the host; gamma stays f32.

Engine placement (per column chunk; measured on HW, see below):
  SP (sync)    : in-DMA triggers ONLY — a dedicated input queue.  Putting
                 out-triggers here head-of-line-blocks the input stream
                 behind the gating multiply (+4..10us measured).
  DVE (vector) : pooling tree (bf16 2x packed) + EMA scans (the scan
                 opcode only exists on DVE) + gating multiply (2x).  This
                 queue paces the kernel end-to-end (~38us busy).
  Act (scalar) : relu, sigmoid, the whole gate expansion (gpsimd copies
                 measured 4x slower than the cost model claims).
  Pool (gpsimd): out-DMA triggers ONLY (SWDGE) — dedicated output queue.
  PE (tensor)  : SE bottleneck matmuls in bf16.
The gating multiply + out-DMA of chunk k are emitted `skew` chunks later
so the DVE queue never stalls waiting on the SE chain, and each chunk is
split in ct-halves (gsplit=2) so outs start as soon as half the gating
is done.

HW-measured context (8 cores concurrent, per core): mixed in+out DMA
streams at ~384 GB/s -> 43.7us floor for 16.8 MB; this kernel runs
~58us single-shot (honest ping-pong-reps timing; the naive same-address
reps measurement dead-store-eliminates the out-DMAs and reads ~15% fast).

Timing-graph liveness: reps>1 graphs "ping-pong" the I/O (rep 0 reads
x writes out, rep 1 reads out writes out2, ...) so every rep's transfers
and compute are data-live — otherwise the compiler dead-store-eliminates
repeated same-address out-DMAs and the reps-slope undercounts.
"""

import numpy as np
from contextlib import ExitStack

import concourse.bass as bass
import concourse.tile as tile
from concourse import bacc, mybir

F32 = mybir.dt.float32
BF16 = mybir.dt.bfloat16
P = 128


def _bcast_ap(ap, n, drop_last=False):
    """Append a stride-0 dim of size n to an AP (optionally replacing a
    trailing size-1 dim)."""
    dims = [list(d) for d in ap.ap]
    if drop_last:
        assert dims[-1][1] == 1, dims
        dims = dims[:-1]
    dims = dims + [[0, n]]
    return bass.AP(tensor=ap.tensor, offset=ap.offset, ap=dims)


def build_graph(C=512, L=8192, CS=16, HID=64, reps=1, chunks=None,
                scan_eng="gpsimd", exp_act=3, out_eng="sync", skew=2,
                gsplit=1, jslab=16, pingpong=False, serialize=False,
                ablate=None, xbufs=1, sbufs=3, ebufs=4, pbufs=2):
    """Build the per-core Bass graph (SPMD: every core runs this same graph).

    chunks : column widths (each a multiple of CS, sum == L).
    exp_act: how many of the NCT channel tiles get their gate expansion on
             the Act engine (the rest go on Pool/gpsimd).
    out_eng: engine issuing the out-DMA triggers ("sync"|"scalar").
    pingpong: reps>1 timing graphs alternate DRAM src/dst so all work is
             live (see module docstring).  reps==1 is the real kernel.
    serialize: chain rep r's first in-DMA after rep r-1's last out-DMA
             (single-shot latency instead of pipelined throughput).
    """
    NCT = C // P
    if chunks is None:
        chunks = [512, 1536, 2048, 2048, 1536, 512]
    assert sum(chunks) == L and all(c % CS == 0 for c in chunks)
    NCmax = max(chunks) // CS
    LCmax = max(chunks)

    nc = bacc.Bacc(None, target_bir_lowering=False)

    x_ext = nc.declare_dram_parameter("x", [C, L], BF16, isOutput=False)
    w1_ext = nc.declare_dram_parameter("w1s", [P, NCT * HID], BF16, isOutput=False)
    w2_ext = nc.declare_dram_parameter("w2t", [HID, C], BF16, isOutput=False)
    b1_ext = nc.declare_dram_parameter("b1", [HID, 1], F32, isOutput=False)
    g_ext = nc.declare_dram_parameter("g", [P, NCT], F32, isOutput=False)
    out_ext = nc.declare_dram_parameter("out", [C, L], BF16, isOutput=True)
    dbg_ext = None
    if ablate == "no_gate":
        # keep the SE chain live without the expansion/multiply consumers
        dbg_ext = nc.declare_dram_parameter(
            "dbg", [P, reps * len(chunks) * NCT * NCmax], BF16, isOutput=True)
    views = [x_ext[:].rearrange("(ct p) l -> p ct l", ct=NCT),
             out_ext[:].rearrange("(ct p) l -> p ct l", ct=NCT)]
    if pingpong and reps > 1:
        out2_ext = nc.declare_dram_parameter("out2", [C, L], BF16, isOutput=True)
        views.append(out2_ext[:].rearrange("(ct p) l -> p ct l", ct=NCT))

    from concourse.tile_rust import add_dep_helper

    with ExitStack() as ctx:
        tc = ctx.enter_context(tile.TileContext(nc))
        consts = ctx.enter_context(tc.tile_pool(name="consts", bufs=1))
        xpool = ctx.enter_context(tc.tile_pool(name="xpool", bufs=xbufs))
        small = ctx.enter_context(tc.tile_pool(name="small", bufs=sbufs))
        epool = ctx.enter_context(tc.tile_pool(name="epool", bufs=ebufs))
        psum = ctx.enter_context(
            tc.tile_pool(name="psum", bufs=pbufs, space=bass.MemorySpace.PSUM)
        )

        # -- head: warm the ACT function tables, load the small constants --
        warm = consts.tile([P, 1], F32, name="warm")
        nc.gpsimd.memset(warm[:], 0.0)
        nc.scalar.activation(out=warm[:], in_=warm[:],
                             func=mybir.ActivationFunctionType.Sigmoid)
        nc.scalar.activation(out=warm[:], in_=warm[:],
                             func=mybir.ActivationFunctionType.Relu)

        g_sb = consts.tile([P, NCT], F32, name="g_sb")
        nc.scalar.dma_start(out=g_sb[:], in_=g_ext[:])
        w1_sb = consts.tile([P, NCT, HID], BF16)
        nc.scalar.dma_start(
            out=w1_sb[:], in_=w1_ext[:].rearrange("p (ct h) -> p ct h", ct=NCT)
        )
        w2_sb = consts.tile([HID, C], BF16)
        nc.scalar.dma_start(out=w2_sb[:], in_=w2_ext[:])
        b1_sb = consts.tile([HID, 1], F32)
        nc.scalar.dma_start(out=b1_sb[:], in_=b1_ext[:])

        # materialized per-ct gamma rows for the scan's data0 operand
        ones = consts.tile([P, NCmax], F32)
        nc.vector.memset(ones[:], 1.0)
        g_bcast = []
        for ct in range(NCT):
            gb = consts.tile([P, NCmax], F32, tag=f"gb{ct}")
            nc.vector.tensor_scalar_mul(gb[:], ones[:], g_sb[:, ct : ct + 1])
            g_bcast.append(gb)

        scan_e = {"gpsimd": nc.gpsimd, "vector": nc.vector}[scan_eng]
        out_engines = {"sync": [nc.sync], "scalar": [nc.scalar],
                       "gpsimd": [nc.gpsimd],
                       "alt": [nc.gpsimd, nc.scalar]}[out_eng]
        out_idx = [0]

        const_gate = None
        if ablate == "no_se":
            const_gate = consts.tile([P, NCT, LCmax], BF16, name="cgate")
            nc.vector.memset(const_gate[:], 0.5)
        dbg_i = 0

        prev_rep_last_out = None
        for _r in range(reps):
            src = views[0] if _r == 0 else views[1 + ((_r - 1) % 2)]
            dst = views[1 + (_r % 2)] if (pingpong and reps > 1) else views[1]
            last_out_inst = None
            # chunk-block-major resident x: each chunk's [ct, LC] block is a
            # CONTIGUOUS column range, so Tile's byte-range hazard tracking
            # sees the chunks as disjoint and the pipeline stays overlapped.
            x_big = xpool.tile([P, NCT * L], BF16, tag="xbig", name="xbig")
            u_prev = None
            prev_nc = 0
            col = 0
            off = 0
            pending = []  # deferred gating multiplies / out-DMAs (skewed)

            def emit_gate_apply(poff, pcol, pLC, gate_exp, jslab):
                nonlocal last_out_inst
                xblk = x_big[:, poff : poff + NCT * pLC].rearrange(
                    "p (ct l) -> p ct l", ct=NCT
                )
                gw = NCT // gsplit
                pNC = pLC // CS
                for g in range(gsplit):
                    c0, c1 = g * gw, (g + 1) * gw
                    if jslab == CS:
                        nc.vector.tensor_tensor(
                            out=xblk[:, c0:c1, :],
                            in0=xblk[:, c0:c1, :],
                            in1=gate_exp[:, c0:c1, :pLC],
                            op=mybir.AluOpType.mult,
                        )
                    else:
                        # gate expanded only to j-slabs of width jslab; the
                        # multiply walks the CS/jslab phases (all 2x packed)
                        x4 = xblk[:, c0:c1, :].rearrange(
                            "p ct (n j) -> p ct n j", j=CS
                        )
                        g4 = gate_exp[:, c0:c1, : pNC * jslab].rearrange(
                            "p ct (n j) -> p ct n j", j=jslab
                        )
                        for j0 in range(0, CS, jslab):
                            nc.vector.tensor_tensor(
                                out=x4[:, :, :, j0 : j0 + jslab],
                                in0=x4[:, :, :, j0 : j0 + jslab],
                                in1=g4,
                                op=mybir.AluOpType.mult,
                            )
                    last_out_inst = out_engines[
                        out_idx[0] % len(out_engines)
                    ].dma_start(
                        out=dst[:, c0:c1, pcol : pcol + pLC],
                        in_=xblk[:, c0:c1, :],
                    )
                    out_idx[0] += 1

            for k, LC in enumerate(chunks):
                NCc = LC // CS
                xblk = x_big[:, off : off + NCT * LC].rearrange(
                    "p (ct l) -> p ct l", ct=NCT
                )
                in_inst = nc.sync.dma_start(
                    out=xblk, in_=src[:, :, col : col + LC]
                )
                if serialize and k == 0 and prev_rep_last_out is not None:
                    add_dep_helper(
                        in_inst.ins,
                        prev_rep_last_out.ins,
                        reason="serialize reps for single-shot timing",
                    )

                if ablate == "no_se":
                    pending.append((off, col, LC, const_gate, CS))
                    if len(pending) > skew:
                        emit_gate_apply(*pending.pop(0))
                    col += LC
                    off += NCT * LC
                    continue

                # fused pooling as a pairwise-halving tree: each stage sums
                # the two halves of every group, with contiguous >=2-element
                # runs so the DVE's 2x packed mode applies.
                G = NCT * NCc
                stage_src = x_big[:, off : off + NCT * LC]
                r = CS
                while r > 1:
                    h = r // 2
                    v = stage_src.rearrange("p (g j) -> p g j", j=r)
                    dt_ = small.tile([P, NCT * NCmax * h], BF16, tag=f"pool{h}")
                    nc.vector.tensor_tensor(
                        out=dt_[:, : G * h].rearrange("p (g j) -> p g j", j=h),
                        in0=v[:, :, :h],
                        in1=v[:, :, h:],
                        op=mybir.AluOpType.add,
                    )
                    stage_src = dt_[:, : G * h]
                    r = h
                xc_all = stage_src.rearrange("p (ct n) -> p ct n", ct=NCT)

                # causal EMA over chunk steps (per ct; serial across chunks)
                u_all = small.tile([P, NCT, NCmax], BF16, tag="u")
                for ct in range(NCT):
                    init = (
                        0.0 if k == 0
                        else u_prev[:, ct, prev_nc - 1 : prev_nc]
                    )
                    scan_e.tensor_tensor_scan(
                        out=u_all[:, ct, :NCc],
                        data0=g_bcast[ct][:, :NCc],
                        data1=xc_all[:, ct, :NCc],
                        initial=init,
                        op0=mybir.AluOpType.mult,
                        op1=mybir.AluOpType.add,
                    )

                # SE bottleneck
                h_ps = psum.tile([HID, NCmax], F32, tag="hps")
                for ct in range(NCT):
                    nc.tensor.matmul(
                        h_ps[:, :NCc],
                        w1_sb[:, ct, :],
                        u_all[:, ct, :NCc],
                        start=(ct == 0),
                        stop=(ct == NCT - 1),
                    )
                h_sb = small.tile([HID, NCmax], BF16, tag="h")
                nc.scalar.activation(
                    out=h_sb[:, :NCc],
                    in_=h_ps[:, :NCc],
                    func=mybir.ActivationFunctionType.Relu,
                    bias=b1_sb[:],
                )
                o_all = psum.tile([P, NCT * NCmax], F32, tag="ops")
                for ct in range(NCT):
                    nc.tensor.matmul(
                        o_all[:, ct * NCc : (ct + 1) * NCc],
                        w2_sb[:, ct * P : (ct + 1) * P],
                        h_sb[:, :NCc],
                        start=True,
                        stop=True,
                    )
                gate_all = small.tile([P, NCT, NCmax], BF16, tag="gate")
                o_v = o_all[:, : NCT * NCc].rearrange("p (ct n) -> p ct n", ct=NCT)
                nc.scalar.activation(
                    out=gate_all[:, :, :NCc],
                    in_=o_v,
                    func=mybir.ActivationFunctionType.Sigmoid,
                )

                if ablate == "no_gate":
                    # ship the gate to a debug output instead of gating
                    nc.scalar.dma_start(
                        out=dbg_ext[:, dbg_i : dbg_i + NCT * NCmax],
                        in_=gate_all[:, :, :].rearrange("p ct n -> p (ct n)"),
                    )
                    dbg_i += NCT * NCmax
                    u_prev = u_all
                    prev_nc = NCc
                    col += LC
                    off += NCT * LC
                    continue

                # expand gate [p, ct, n] -> [p, ct, n*jslab] (j-slab; the
                # multiply re-reads the slab for each of the CS/jslab phases)
                js_k = jslab[k] if isinstance(jslab, (list, tuple)) else jslab
                js_max = max(jslab) if isinstance(jslab, (list, tuple)) else jslab
                gate_exp = epool.tile([P, NCT, NCmax * js_max], BF16, tag="gexp")
                if gsplit > 1:
                    gw = NCT // gsplit
                    for g in range(gsplit):
                        c0, c1 = g * gw, (g + 1) * gw
                        nc.scalar.copy(
                            out=gate_exp[:, c0:c1, : NCc * js_k].rearrange(
                                "p ct (n j) -> p ct n j", j=js_k
                            ),
                            in_=_bcast_ap(gate_all[:, c0:c1, :NCc], js_k),
                        )
                else:
                    na = min(exp_act, NCT)
                    if na > 0:
                        nc.scalar.copy(
                            out=gate_exp[:, :na, :LC].rearrange(
                                "p ct (n j) -> p ct n j", j=CS
                            ),
                            in_=_bcast_ap(gate_all[:, :na, :NCc], CS),
                        )
                    if na < NCT:
                        nc.gpsimd.tensor_copy(
                            gate_exp[:, na:, :LC].rearrange(
                                "p ct (n j) -> p ct n j", j=CS
                            ),
                            _bcast_ap(gate_all[:, na:, :NCc], CS),
                        )

                pending.append((off, col, LC, gate_exp, js_k))
                skew_k = skew[k] if isinstance(skew, (list, tuple)) else skew
                while len(pending) > skew_k:
                    emit_gate_apply(*pending.pop(0))

                u_prev = u_all
                prev_nc = NCc
                col += LC
                off += NCT * LC

            while pending:
                emit_gate_apply(*pending.pop(0))
            prev_rep_last_out = last_out_inst

    nc.compile()
    return nc


def _f32_to_bf16(a):
    """Round-to-nearest-even f32 -> bf16, vectorized (no NaN handling)."""
    import ml_dtypes

    u = np.ascontiguousarray(a, np.float32).view(np.uint32)
    r = ((u + 0x7FFF + ((u >> 16) & 1)) >> 16).astype(np.uint16)
    return r.view(ml_dtypes.bfloat16)


def _bf16_to_f32(a):
    u = np.asarray(a).view(np.uint16).astype(np.uint32) << 16
    return u.view(np.float32)


def host_prep(gamma, w1, b1, w2, b2, C=512, HID=64):
    """Host-side preprocessing of the shared (small) tensors."""
    NCT = C // P
    gamma = np.asarray(gamma, np.float32)
    w1 = np.asarray(w1, np.float32)
    w2 = np.asarray(w2, np.float32)
    bv = (1.0 - gamma) / 16.0
    w1s = (w1 * bv[None, :]).T  # [C, HID]
    w1s_r = _f32_to_bf16(
        np.ascontiguousarray(
            w1s.reshape(NCT, P, HID).transpose(1, 0, 2).reshape(P, NCT * HID)
        )
    )
    w2t = _f32_to_bf16(np.ascontiguousarray(w2.T))  # [HID, C]
    b1_r = np.ascontiguousarray(np.asarray(b1, np.float32).reshape(HID, 1))
    g_r = np.ascontiguousarray(gamma.reshape(NCT, P).T)
    return w1s_r, w2t, b1_r, g_r


DEFAULT_CFG = dict(
    chunks=[1024, 2048, 2048, 2048, 768, 256],
    scan_eng="vector",
    exp_act=4,
    out_eng="gpsimd",
    # chunk k's gating multiply + out-DMA are deferred until the pending
    # queue exceeds skew[k]: chunk 0 flushes right after chunk 1 (its
    # expansion is ready by then with jslab=4, filling the DVE input-wait
    # gap at the head), later chunks run 2 behind.
    skew=[9, 1, 2, 2, 2, 2],
    gsplit=2,
    # gate expanded only to j-slabs (the multiply walks CS/jslab phases,
    # all 2x packed), cutting the 27us Act expansion pass several-fold.
    # Narrow slabs (4) at the head/tail where expansion latency gates the
    # early-popped / flushed multiplies; wider (8) mid-stream where the
    # latency is hidden and the halved DVE mult-op count matters more.
    jslab=[4, 8, 8, 8, 8, 4],
    ebufs=5,
)

_GRAPH_CACHE = {}


def _get_graph(reps=1):
    key = reps
    if key not in _GRAPH_CACHE:
        _GRAPH_CACHE[key] = build_graph(reps=reps, **DEFAULT_CFG)
    return _GRAPH_CACHE[key]


def make_in_maps(x, gamma, w1, b1, w2, b2):
    B, C, L = x.shape
    HID = w1.shape[0]
    w1s_r, w2t, b1_r, g_r = host_prep(gamma, w1, b1, w2, b2, C=C, HID=HID)
    xb = _f32_to_bf16(x).reshape(B, C, L)
    return [
        {
            "x": xb[b],  # view of the contiguous parent -> no copy downstream
            "w1s": w1s_r,
            "w2t": w2t,
            "b1": b1_r,
            "g": g_r,
        }
        for b in range(B)
    ]


_RUNNER_CACHE = {}


def _make_runner(nc, n_cores):
    """Persistent jitted SPMD runner for `nc` across `n_cores` devices.

    Returns run(in_maps) -> list[dict] of per-core outputs.
    """
    import jax
    from jax.sharding import Mesh, PartitionSpec
    from jax.experimental.shard_map import shard_map
    from concourse import bass2jax

    bass2jax.install_neuronx_cc_hook()

    partition_name = nc.partition_id_tensor.name if nc.partition_id_tensor else None
    in_names, out_names, out_avals = [], [], []
    for alloc in nc.m.functions[0].allocations:
        if not isinstance(alloc, mybir.MemoryLocationSet):
            continue
        name = alloc.memorylocations[0].name
        if alloc.kind == "ExternalInput":
            if name != partition_name:
                in_names.append(name)
        elif alloc.kind == "ExternalOutput":
            out_names.append(name)
            out_avals.append(
                jax.core.ShapedArray(tuple(alloc.tensor_shape), mybir.dt.np(alloc.dtype))
            )
    n_params = len(in_names)
    in_names_all = in_names + out_names
    if partition_name is not None:
        in_names_all.append(partition_name)

    def _body(*args):
        operands = list(args)
        if partition_name is not None:
            operands.append(bass2jax.partition_id_tensor())
        outs = bass2jax._bass_exec_p.bind(
            *operands,
            out_avals=tuple(out_avals),
            in_names=tuple(in_names_all),
            out_names=tuple(out_names),
            lowering_input_output_aliases=(),
            sim_require_finite=True,
            sim_require_nnan=True,
            nc=nc,
        )
        return tuple(outs)

    devices = jax.devices()[:n_cores]
    mesh = Mesh(np.asarray(devices), ("core",))
    n_outs = len(out_avals)
    sharded = jax.jit(
        shard_map(
            _body,
            mesh=mesh,
            in_specs=(PartitionSpec("core"),) * (n_params + n_outs),
            out_specs=(PartitionSpec("core"),) * len(out_names),
            check_rep=False,
        ),
        keep_unused=True,
    )
    concat_zeros = [
        np.zeros((n_cores * a.shape[0], *a.shape[1:]), a.dtype) for a in out_avals
    ]

    def _concat_inputs(in_maps):
        concat_in = []
        for name in in_names:
            parts = [np.asarray(m[name]) for m in in_maps]
            base = parts[0].base if parts[0].base is not None else parts[0]
            if (
                base.ndim == parts[0].ndim + 1
                and base.shape[0] == n_cores
                and base.flags.c_contiguous
                and all(
                    p.base is base
                    and p.__array_interface__["data"][0]
                    == base.__array_interface__["data"][0] + c * parts[0].nbytes
                    for c, p in enumerate(parts)
                )
            ):
                # per-core slices of one contiguous parent: reshape, no copy
                concat_in.append(
                    np.ascontiguousarray(base).reshape(
                        n_cores * parts[0].shape[0], *parts[0].shape[1:]
                    )
                )
            else:
                concat_in.append(np.concatenate(parts, axis=0))
        return concat_in

    def run(in_maps):
        out_arrs = sharded(*_concat_inputs(in_maps), *concat_zeros)
        return [
            {
                name: np.asarray(out_arrs[i]).reshape(
                    n_cores, *out_avals[i].shape
                )[c]
                for i, name in enumerate(out_names)
            }
            for c in range(n_cores)
        ]

    def run_full(in_maps):
        """Like run() but returns the first output as one stacked array
        [n_cores, ...] with a single host copy."""
        out_arrs = sharded(*_concat_inputs(in_maps), *concat_zeros)
        return np.asarray(out_arrs[0]).reshape(n_cores, *out_avals[0].shape)

    run.run_full = run_full
    return run


def _get_runner(reps=1, n_cores=8):
    key = (reps, n_cores)
    if key not in _RUNNER_CACHE:
        _RUNNER_CACHE[key] = _make_runner(_get_graph(reps=reps), n_cores)
    return _RUNNER_CACHE[key]


def kernel(x, gamma, w1, b1, w2, b2):
    x = np.asarray(x)
    B, C, L = x.shape
    assert (B, C, L) == (8, 512, 8192), (B, C, L)
    in_maps = make_in_maps(x, gamma, w1, b1, w2, b2)
    try:
        out = _get_runner(reps=1, n_cores=B).run_full(in_maps)
        return np.ascontiguousarray(_bf16_to_f32(out))
    except Exception:
        # fallback: the official (slower to dispatch, identical NEFF) path
        from concourse.bass_utils import run_bass_kernel_spmd

        res = run_bass_kernel_spmd(
            _get_graph(reps=1), in_maps, core_ids=list(range(B))
        ).results
        out = np.stack([res[b]["out"] for b in range(B)], axis=0)
        return np.ascontiguousarray(_bf16_to_f32(out))
